# revision 15
# baseline (speedup 1.0000x reference)
import sys
if '/opt/trn_rl_repo' not in sys.path:
    sys.path.insert(0, '/opt/trn_rl_repo')
import numpy as np

B, S, D, H, DH, F = 2, 2048, 1024, 16, 64, 4096
NQ = 512           # queries per core
NCORES = 8
P = 128
EPS = 1e-5
VW = 65            # per-head width in the v layout (64 v cols + 1 deadmask col)
VT = 16 * VW       # cols per token-tile in vv = 1040
VPAD = 63          # margin so the odd-h15/kt15 junk group stays in bounds


def build_nc():
    import concourse.bass as bass
    import concourse.tile as tile
    from concourse import bacc, mybir

    f32 = mybir.dt.float32
    f32r = mybir.dt.float32r
    AF = mybir.ActivationFunctionType
    OP = mybir.AluOpType

    def r(ap):
        return ap

    nc = bacc.Bacc("TRN2", target_bir_lowering=False, debug=False,
                   num_devices=NCORES)

    def din(name, shape, dt=f32):
        return nc.dram_tensor(name, shape, dt, kind="ExternalInput").ap()

    xkvT_d = din("xkvT", [D, S], f32r)
    xqT_d = din("xqT", [D, NQ])
    wq_d = din("wq", [D, D], f32r)
    wk_d = din("wk", [D, D], f32r)
    wv_d = din("wv", [D, D], f32r)
    wo_d = din("wo", [D, D], f32r)
    w1_d = din("w1", [D, F], f32r)
    w2_d = din("w2", [F, D], f32r)
    bq_d = din("bq_pt", [P, 8])
    bk_d = din("bk_pt", [P, 8])
    bv_d = din("bv_pt", [P, 8])
    bo_d = din("bo_pt", [P, 8])
    b1_d = din("b1_pt", [P, 32])
    b2_d = din("b2_pt", [P, 8])
    g1_d = din("g1_pt", [P, 8])
    be1_d = din("beta1_pt", [P, 8])
    g2_d = din("g2_pt", [P, 8])
    be2_d = din("beta2_pt", [P, 8])
    tri_d = din("trimask", [P, 4 * NQ], f32r)
    ones_d = din("onesmat", [P, P], f32r)
    dead_d = din("deadmask", [P, 16])
    outT_d = nc.dram_tensor("outT", [D, NQ], f32r, kind="ExternalOutput").ap()

    def load_fp(dst, src, f):
        # DRAM [f*128, C] -> sbuf [128, f*C] chunk-major (dst[p, i*C+c] = src[i*128+p, c])
        c = src.shape[1]
        nc.sync.dma_start(dst.rearrange("p (f c) -> p f c", f=f),
                          src.rearrange("(f p) c -> p f c", p=P))

    with tile.TileContext(nc) as tc:
        import contextlib
        with contextlib.ExitStack() as top:
            persist = top.enter_context(tc.tile_pool(name="persist", bufs=1))
            ones = persist.tile([P, P], f32r)
            nc.sync.dma_start(ones[:], ones_d[:])
            dead = persist.tile([P, 16], f32)
            nc.sync.dma_start(dead[:], dead_d[:])
            biases = {}
            for nm, dr, w in [("bq", bq_d, 8), ("bk", bk_d, 8), ("bv", bv_d, 8),
                              ("bo", bo_d, 8), ("b1", b1_d, 32), ("b2", b2_d, 8),
                              ("g1", g1_d, 8), ("be1", be1_d, 8),
                              ("g2", g2_d, 8), ("be2", be2_d, 8)]:
                t = persist.tile([P, w], f32, name=f"bias_{nm}")
                nc.sync.dma_start(t[:], dr[:])
                biases[nm] = t

            # outT (attention output, transposed, [feat, tok]) survives into phase 3
            outp = top.enter_context(tc.tile_pool(name="outp", bufs=1))
            outT = outp.tile([P, 8 * NQ], f32r)

            with contextlib.ExitStack() as ascope:
                st = ascope.enter_context(
                    tc.tile_pool(name="attn_state", bufs=1))
                kT = st.tile([P, 8 * S], f32r)            # [feat, key] 8 chunks
                qT = st.tile([P, 8 * NQ], f32r)           # [feat, query]

                # ---- phase 1a: kT + qT (x resident, weights streamed once) ----
                with tc.tile_pool(name="xres_p", bufs=1) as xrp, \
                     tc.tile_pool(name="w1a_p", bufs=3) as wp, \
                     tc.tile_pool(name="ps1a", bufs=4, space="PSUM") as pp:
                    xres = xrp.tile([P, 8 * S], f32r)
                    for f in range(8):
                        nc.sync.dma_start(xres[:, f * S:(f + 1) * S],
                                          xkvT_d[f * P:(f + 1) * P, :])
                    for m in range(8):
                        wkt = wp.tile([P, 8 * P], f32r, name="wkt", tag="w1a")
                        load_fp(wkt[:], wk_d[:, m * P:(m + 1) * P], 8)
                        wqt = wp.tile([P, 8 * P], f32r, name="wqt", tag="w1a")
                        load_fp(wqt[:], wq_d[:, m * P:(m + 1) * P], 8)
                        for tq in range(4):
                            ps = pp.tile([P, NQ], f32, name="psk")
                            for f in range(8):
                                nc.tensor.matmul(
                                    ps[:], r(wkt[:, f * P:(f + 1) * P]),
                                    r(xres[:, f * S + tq * NQ: f * S + (tq + 1) * NQ]),
                                    start=(f == 0), stop=(f == 7))
                            nc.scalar.activation(
                                kT[:, m * S + tq * NQ: m * S + (tq + 1) * NQ],
                                ps[:], AF.Identity, bias=biases["bk"][:, m:m + 1])
                        ps = pp.tile([P, NQ], f32, name="psq")
                        for f in range(8):
                            nc.tensor.matmul(
                                ps[:], r(wqt[:, f * P:(f + 1) * P]),
                                r(xres[:, f * S: f * S + NQ]),
                                start=(f == 0), stop=(f == 7))
                        nc.scalar.activation(
                            qT[:, m * NQ:(m + 1) * NQ], ps[:],
                            AF.Identity, bias=biases["bq"][:, m:m + 1])

                # ---- phase 1b: v token-major w/ deadmask, head-interleaved ----
                st2 = ascope.enter_context(tc.tile_pool(name="attn_state2", bufs=1))
                vv = st2.tile([P, 16 * VT + VPAD], f32r)  # [key(tile), head-major v]
                with tc.tile_pool(name="wv_p", bufs=1) as wvp, \
                     tc.tile_pool(name="xb_p", bufs=3) as xbp, \
                     tc.tile_pool(name="ps1b", bufs=4, space="PSUM") as pp:
                    nc.vector.tensor_scalar(vv[:, 16 * VT:16 * VT + VPAD],
                                            ones[:, 0:VPAD], 0.0, None, OP.mult)
                    for tt in range(16):
                        # deadmask columns for this token tile (col 64+65*h)
                        dcols = bass.AP(vv.tensor,
                                        vv.offset + tt * VT + 64,
                                        [list(vv.ap[0]), [VW, 16]])
                        nc.vector.tensor_scalar(
                            dcols, ones[:, 0:16], dead[:, tt:tt + 1], None,
                            OP.mult)
                    for nh in range(2):
                        wvh = wvp.tile([P, 8 * NQ], f32r, name="wvh")
                        load_fp(wvh[:], wv_d[:, nh * NQ:(nh + 1) * NQ], 8)
                        for tt in range(16):
                            xset = xbp.tile([P, 8 * P], f32r, name="xset")
                            load_fp(xset[:], xkvT_d[:, tt * P:(tt + 1) * P], 8)
                            ps = pp.tile([P, NQ], f32, name="psv")
                            for f in range(8):
                                nc.tensor.matmul(
                                    ps[:], r(xset[:, f * P:(f + 1) * P]),
                                    r(wvh[:, f * NQ:(f + 1) * NQ]),
                                    start=(f == 0), stop=(f == 7))
                            # evict with deadmask, strided per-head layout
                            # psum col c (feat nh*512+c) -> vv col 65*(c//64)+(c%64)
                            vap = bass.AP(vv.tensor,
                                          vv.offset + tt * VT + nh * 8 * VW,
                                          [list(vv.ap[0]), [VW, 8], [1, 64]])
                            nc.vector.tensor_scalar(
                                vap, ps[:], dead[:, tt:tt + 1], None, OP.mult)

                # ---- phase 2: attention ----
                with tc.tile_pool(name="attn_p", bufs=4) as apool, \
                     tc.tile_pool(name="tri_p", bufs=1) as trip, \
                     tc.tile_pool(name="rec_p", bufs=2) as rpool, \
                     tc.tile_pool(name="ps2s", bufs=2, space="PSUM") as pps, \
                     tc.tile_pool(name="ps2a", bufs=2, space="PSUM") as ppa, \
                     tc.tile_pool(name="ps2b", bufs=2, space="PSUM") as ppb:
                    tri = trip.tile([P, 4 * NQ], f32r)
                    nc.sync.dma_start(tri[:], tri_d[:])
                    for h in range(H):
                        par, ht = h % 2, h // 2
                        pb = par * 64
                        ps_av = ppa.tile([P, NQ], f32, name="ps_av")
                        for kt in range(16):
                            ps_s = pps.tile([P, NQ], f32, name="ps_s")
                            nc.tensor.matmul(
                                ps_s[:],
                                r(kT[pb:pb + 64, ht * S + kt * P: ht * S + (kt + 1) * P]),
                                r(qT[pb:pb + 64, ht * NQ:(ht + 1) * NQ]),
                                start=True, stop=True)
                            attn = apool.tile([P, NQ], f32r, name="attn")
                            nc.scalar.activation(attn[:], ps_s[:], AF.Exp,
                                                 scale=0.125)
                            if kt < 4:
                                nc.vector.tensor_mul(
                                    attn[:], attn[:],
                                    tri[:, kt * NQ:(kt + 1) * NQ])
                            lhs = vv[:, kt * VT + VW * h: kt * VT + VW * h + VW]
                            nc.tensor.matmul(ps_av[0:VW, :], r(lhs), r(attn[:]),
                                             start=(kt == 0), stop=(kt == 15),
                                             skip_group_check=True)
                        # data rows 0..63, sumexp row 64 (all heads); odd heads
                        # are placed at outT rows 64..127 via a shifted DVE write
                        recip = rpool.tile([P, NQ], f32r, name="recip")
                        with nc.allow_low_precision(reason="f32r==f32 bits"):
                            nc.vector.reciprocal(recip[64:65, :],
                                                 ps_av[64:65, :])
                        ps_b = ppb.tile([P, NQ], f32, name="ps_b")
                        nc.tensor.matmul(ps_b[0:64, :],
                                         r(ones[64:65, 0:64]),
                                         r(recip[64:65, :]),
                                         start=True, stop=True)
                        rb = rpool.tile([P, NQ], f32, name="rb", tag="rb")
                        nc.scalar.copy(rb[0:64, :], ps_b[0:64, :])
                        oslice = outT[pb:pb + 64, ht * NQ:(ht + 1) * NQ]
                        nc.vector.tensor_mul(oslice, ps_av[0:64, :],
                                             rb[0:64, :])
                        nc.vector.tensor_scalar(
                            oslice, oslice,
                            biases["bv"][pb:pb + 64, h // 2: h // 2 + 1],
                            None, OP.add)

            # ---- phase 3: O-proj + LN1 + FFN + LN2 ----
            with tc.tile_pool(name="f3", bufs=1) as f3, \
                 tc.tile_pool(name="w3_p", bufs=2) as wp3, \
                 tc.tile_pool(name="sq_p", bufs=2) as sqp, \
                 tc.tile_pool(name="stat_p", bufs=2) as stp, \
                 tc.tile_pool(name="psmm", bufs=2, space="PSUM") as pmm, \
                 tc.tile_pool(name="psst", bufs=2, space="PSUM") as pst, \
                 tc.tile_pool(name="psbc", bufs=2, space="PSUM") as pbc:
                xq = f3.tile([P, 8 * NQ], f32)
                for m in range(8):
                    nc.sync.dma_start(xq[:, m * NQ:(m + 1) * NQ],
                                      xqT_d[m * P:(m + 1) * P, :])
                x1 = f3.tile([P, 8 * NQ], f32r)
                hh = f3.tile([P, 32 * NQ], f32r)
                x2 = f3.tile([P, 8 * NQ], f32r)

                # O-projection + residual
                for m in range(8):
                    wot = wp3.tile([P, 8 * P], f32r, name="wot", tag="wsmall")
                    load_fp(wot[:], wo_d[:, m * P:(m + 1) * P], 8)
                    ps = pmm.tile([P, NQ], f32, name="psmm")
                    for f2 in range(8):
                        nc.tensor.matmul(
                            ps[:], r(wot[:, f2 * P:(f2 + 1) * P]),
                            r(outT[:, f2 * NQ:(f2 + 1) * NQ]),
                            start=(f2 == 0), stop=(f2 == 7))
                    nc.vector.scalar_tensor_tensor(
                        x1[:, m * NQ:(m + 1) * NQ], ps[:],
                        biases["bo"][:, m:m + 1],
                        xq[:, m * NQ:(m + 1) * NQ], OP.add, OP.add)

                def layer_norm(xt, gname, bname):
                    ps_sum = pst.tile([1, NQ], f32, name="ps_sum", tag="sum")
                    ps_sq = pst.tile([1, NQ], f32, name="ps_sq", tag="sumsq")
                    for m in range(8):
                        nc.tensor.matmul(ps_sum[:], r(ones[:, 0:1]),
                                         r(xt[:, m * NQ:(m + 1) * NQ]),
                                         start=(m == 0), stop=(m == 7),
                                         skip_group_check=True)
                        sq = sqp.tile([P, NQ], f32r, name="sq")
                        nc.scalar.activation(sq[:], xt[:, m * NQ:(m + 1) * NQ],
                                             AF.Square)
                        nc.tensor.matmul(ps_sq[:], r(ones[:, 0:1]), r(sq[:]),
                                         start=(m == 0), stop=(m == 7),
                                         skip_group_check=True)
                    mu = stp.tile([1, NQ], f32r, name="mu")
                    nc.scalar.activation(mu[:], ps_sum[:], AF.Copy,
                                         scale=1.0 / D)
                    ex2 = stp.tile([1, NQ], f32, name="ex2")
                    nc.scalar.activation(ex2[:], ps_sq[:], AF.Copy,
                                         scale=1.0 / D)
                    var = stp.tile([1, NQ], f32, name="var")
                    nc.vector.scalar_tensor_tensor(
                        var[:], mu[:], 0.0, mu[:], OP.add, OP.mult)
                    # var = (ex2 + EPS) - mu^2
                    nc.vector.scalar_tensor_tensor(
                        var[:], ex2[:], EPS, var[:], OP.add, OP.subtract)
                    std = stp.tile([1, NQ], f32, name="std")
                    nc.scalar.activation(std[:], var[:], AF.Sqrt, bias=0.0)
                    rstd = stp.tile([1, NQ], f32r, name="rstd")
                    with nc.allow_low_precision(reason="f32r==f32 bits"):
                        nc.vector.reciprocal(rstd[:], std[:])
                    ps_mu = pbc.tile([P, NQ], f32, name="ps_mu", tag="bc")
                    nc.tensor.matmul(ps_mu[:], r(ones[0:1, :]), r(mu[:]),
                                     start=True, stop=True)
                    ps_rs = pbc.tile([P, NQ], f32, name="ps_rs", tag="bc")
                    nc.tensor.matmul(ps_rs[:], r(ones[0:1, :]), r(rstd[:]),
                                     start=True, stop=True)
                    for m in range(8):
                        sl = xt[:, m * NQ:(m + 1) * NQ]
                        nc.vector.tensor_sub(sl, sl, ps_mu[:])
                        nc.vector.tensor_mul(sl, sl, ps_rs[:])
                        nc.scalar.activation(sl, sl, AF.Identity,
                                             bias=biases[bname][:, m:m + 1],
                                             scale=biases[gname][:, m:m + 1])

                layer_norm(x1, "g1", "be1")

                # FFN1 (relu) -> hh
                for mf in range(32):
                    w1t = wp3.tile([P, 8 * P], f32r, name="w1t", tag="wsmall")
                    load_fp(w1t[:], w1_d[:, mf * P:(mf + 1) * P], 8)
                    ps = pmm.tile([P, NQ], f32, name="psmm")
                    for f in range(8):
                        nc.tensor.matmul(
                            ps[:], r(w1t[:, f * P:(f + 1) * P]),
                            r(x1[:, f * NQ:(f + 1) * NQ]),
                            start=(f == 0), stop=(f == 7))
                    nc.scalar.activation(hh[:, mf * NQ:(mf + 1) * NQ], ps[:],
                                         AF.Relu, bias=biases["b1"][:, mf:mf + 1])

                # FFN2 + residual -> x2
                for m in range(8):
                    w2t = wp3.tile([P, 32 * P], f32r, name="w2t", tag="wbig")
                    load_fp(w2t[:], w2_d[:, m * P:(m + 1) * P], 32)
                    ps = pmm.tile([P, NQ], f32, name="psmm")
                    for kf in range(32):
                        nc.tensor.matmul(
                            ps[:], r(w2t[:, kf * P:(kf + 1) * P]),
                            r(hh[:, kf * NQ:(kf + 1) * NQ]),
                            start=(kf == 0), stop=(kf == 31))
                    nc.vector.scalar_tensor_tensor(
                        x2[:, m * NQ:(m + 1) * NQ], ps[:],
                        biases["b2"][:, m:m + 1],
                        x1[:, m * NQ:(m + 1) * NQ], OP.add, OP.add)

                layer_norm(x2, "g2", "be2")

                for m in range(8):
                    nc.sync.dma_start(outT_d[m * P:(m + 1) * P, :],
                                      x2[:, m * NQ:(m + 1) * NQ])

    nc.compile()
    return nc


_CACHE = {}


def make_in_maps(inputs):
    x = np.asarray(inputs['x'], dtype=np.float32)
    shared = {}
    for nm in ("wq", "wk", "wv", "wo", "w1", "w2"):
        shared[nm] = np.ascontiguousarray(np.asarray(inputs[nm], np.float32))
    for nm, w in [("bq", 8), ("bk", 8), ("bv", 8), ("bo", 8), ("b2", 8)]:
        shared[nm + "_pt"] = np.ascontiguousarray(
            np.asarray(inputs[nm], np.float32).reshape(w, P).T)
    shared["b1_pt"] = np.ascontiguousarray(
        np.asarray(inputs["b1"], np.float32).reshape(32, P).T)
    for src, dst in [("g1", "g1_pt"), ("beta1", "beta1_pt"),
                     ("g2", "g2_pt"), ("beta2", "beta2_pt")]:
        shared[dst] = np.ascontiguousarray(
            np.asarray(inputs[src], np.float32).reshape(8, P).T)
    shared["onesmat"] = np.ones((P, P), np.float32)
    tri = np.zeros((P, 4 * NQ), np.float32)
    for kt in range(4):
        k_loc = kt * P + np.arange(P)[:, None]
        q = np.arange(NQ)[None, :]
        tri[:, kt * NQ:(kt + 1) * NQ] = (k_loc <= q).astype(np.float32)
    shared["trimask"] = tri

    in_maps = []
    for c in range(NCORES):
        b, j = c // 4, c % 4
        xb = x[b]                                  # [S, D]
        perm = np.concatenate([np.arange(NQ * j, NQ * (j + 1)),
                               np.arange(0, NQ * j),
                               np.arange(NQ * (j + 1), S)])
        m = dict(shared)
        m["xkvT"] = np.ascontiguousarray(xb.T[:, perm])
        m["xqT"] = np.ascontiguousarray(xb.T[:, NQ * j:NQ * (j + 1)])
        ndead = S - (NQ + NQ * j)
        dead = np.concatenate([np.ones(NQ + NQ * j, np.float32),
                               np.zeros(ndead, np.float32)])
        m["deadmask"] = np.ascontiguousarray(dead.reshape(16, P).T)
        in_maps.append(m)
    return in_maps


def kernel(**inputs):
    from concourse.bass_utils import run_bass_kernel_spmd
    if "nc" not in _CACHE:
        _CACHE["nc"] = build_nc()
    nc = _CACHE["nc"]
    in_maps = make_in_maps(inputs)
    res = run_bass_kernel_spmd(nc, in_maps, core_ids=list(range(NCORES)))
    out = np.empty((B, S, D), np.float32)
    for c in range(NCORES):
        b, j = c // 4, c % 4
        out[b, NQ * j:NQ * (j + 1), :] = res.results[c]["outT"].T
    return out


# revision 17
# speedup vs baseline: 1.1395x; 1.1395x over previous
import sys
if '/opt/trn_rl_repo' not in sys.path:
    sys.path.insert(0, '/opt/trn_rl_repo')
import numpy as np

B, S, D, H, DH, F = 2, 2048, 1024, 16, 64, 4096
NQ = 512           # queries per core
NCORES = 8
P = 128
EPS = 1e-5
VW = 65            # per-head width in the v layout (64 v cols + 1 deadmask col)
VT = 16 * VW       # cols per token-tile in vv = 1040
VPAD = 63          # margin so the odd-h15/kt15 junk group stays in bounds


def build_nc():
    import concourse.bass as bass
    import concourse.tile as tile
    from concourse import bacc, mybir

    f32 = mybir.dt.float32
    f32r = mybir.dt.float32r
    bf16 = mybir.dt.bfloat16
    AF = mybir.ActivationFunctionType
    OP = mybir.AluOpType

    def r(ap):
        return ap

    nc = bacc.Bacc("TRN2", target_bir_lowering=False, debug=False,
                   num_devices=NCORES)

    def din(name, shape, dt=f32):
        return nc.dram_tensor(name, shape, dt, kind="ExternalInput").ap()

    xkvT_d = din("xkvT", [D, S], f32r)
    xqT_d = din("xqT", [D, NQ])
    wq_d = din("wq", [D, D], f32r)
    wk_d = din("wk", [D, D], f32r)
    wv_d = din("wv", [D, D], bf16)
    xkvbf_d = din("xkv_bf", [D, S], bf16)
    wo_d = din("wo", [D, D], f32r)
    w1_d = din("w1", [D, F], f32r)
    w2_d = din("w2", [F, D], f32r)
    bq_d = din("bq_pt", [P, 8])
    bk_d = din("bk_pt", [P, 8])
    bv_d = din("bv_pt", [P, 8])
    bo_d = din("bo_pt", [P, 8])
    b1_d = din("b1_pt", [P, 32])
    b2_d = din("b2_pt", [P, 8])
    g1_d = din("g1_pt", [P, 8])
    be1_d = din("beta1_pt", [P, 8])
    g2_d = din("g2_pt", [P, 8])
    be2_d = din("beta2_pt", [P, 8])
    tri_d = din("trimask", [P, 4 * NQ], bf16)
    ones_d = din("onesmat", [P, P], f32r)
    onesbf_d = din("onesbf", [P, P], bf16)
    dead_d = din("deadmask", [P, 16])
    outT_d = nc.dram_tensor("outT", [D, NQ], f32r, kind="ExternalOutput").ap()

    def load_fp(dst, src, f):
        # DRAM [f*128, C] -> sbuf [128, f*C] chunk-major (dst[p, i*C+c] = src[i*128+p, c])
        c = src.shape[1]
        nc.sync.dma_start(dst.rearrange("p (f c) -> p f c", f=f),
                          src.rearrange("(f p) c -> p f c", p=P))

    with tile.TileContext(nc) as tc:
        import contextlib
        with contextlib.ExitStack() as top:
            persist = top.enter_context(tc.tile_pool(name="persist", bufs=1))
            ones = persist.tile([P, P], f32r)
            nc.sync.dma_start(ones[:], ones_d[:])
            onesbf = persist.tile([P, P], bf16)
            nc.sync.dma_start(onesbf[:], onesbf_d[:])
            dead = persist.tile([P, 16], f32)
            nc.sync.dma_start(dead[:], dead_d[:])
            biases = {}
            for nm, dr, w in [("bq", bq_d, 8), ("bk", bk_d, 8), ("bv", bv_d, 8),
                              ("bo", bo_d, 8), ("b1", b1_d, 32), ("b2", b2_d, 8),
                              ("g1", g1_d, 8), ("be1", be1_d, 8),
                              ("g2", g2_d, 8), ("be2", be2_d, 8)]:
                t = persist.tile([P, w], f32, name=f"bias_{nm}")
                nc.sync.dma_start(t[:], dr[:])
                biases[nm] = t

            # outT (attention output, transposed, [feat, tok]) survives into phase 3
            outp = top.enter_context(tc.tile_pool(name="outp", bufs=1))
            outT = outp.tile([P, 8 * NQ], f32r)

            with contextlib.ExitStack() as ascope:
                st = ascope.enter_context(
                    tc.tile_pool(name="attn_state", bufs=1))
                kT = st.tile([P, 8 * S], bf16)            # [feat, key] 8 chunks
                qT = st.tile([P, 8 * NQ], bf16)           # [feat, query]

                # ---- phase 1a: kT + qT (x resident, weights streamed once) ----
                with tc.tile_pool(name="xres_p", bufs=1) as xrp, \
                     tc.tile_pool(name="w1a_p", bufs=3) as wp, \
                     tc.tile_pool(name="ps1a", bufs=4, space="PSUM") as pp:
                    xres = xrp.tile([P, 8 * S], f32r)
                    for f in range(8):
                        nc.sync.dma_start(xres[:, f * S:(f + 1) * S],
                                          xkvT_d[f * P:(f + 1) * P, :])
                    for m in range(8):
                        wkt = wp.tile([P, 8 * P], f32r, name="wkt", tag="w1a")
                        load_fp(wkt[:], wk_d[:, m * P:(m + 1) * P], 8)
                        wqt = wp.tile([P, 8 * P], f32r, name="wqt", tag="w1a")
                        load_fp(wqt[:], wq_d[:, m * P:(m + 1) * P], 8)
                        for tq in range(4):
                            ps = pp.tile([P, NQ], f32, name="psk")
                            for f in range(8):
                                nc.tensor.matmul(
                                    ps[:], r(wkt[:, f * P:(f + 1) * P]),
                                    r(xres[:, f * S + tq * NQ: f * S + (tq + 1) * NQ]),
                                    start=(f == 0), stop=(f == 7))
                            nc.scalar.activation(
                                kT[:, m * S + tq * NQ: m * S + (tq + 1) * NQ],
                                ps[:], AF.Identity, bias=biases["bk"][:, m:m + 1])
                        ps = pp.tile([P, NQ], f32, name="psq")
                        for f in range(8):
                            nc.tensor.matmul(
                                ps[:], r(wqt[:, f * P:(f + 1) * P]),
                                r(xres[:, f * S: f * S + NQ]),
                                start=(f == 0), stop=(f == 7))
                        nc.scalar.activation(
                            qT[:, m * NQ:(m + 1) * NQ], ps[:],
                            AF.Identity, bias=biases["bq"][:, m:m + 1])

                # ---- phase 1b: v token-major w/ deadmask, head-interleaved ----
                st2 = ascope.enter_context(tc.tile_pool(name="attn_state2", bufs=1))
                vv = st2.tile([P, 16 * VT + VPAD], bf16)  # [key(tile), head-major v]
                with tc.tile_pool(name="wv_p", bufs=2) as wvp, \
                     tc.tile_pool(name="xb_p", bufs=3) as xbp, \
                     tc.tile_pool(name="ps1b", bufs=4, space="PSUM") as pp:
                    nc.vector.tensor_scalar(vv[:, 16 * VT:16 * VT + VPAD],
                                            onesbf[:, 0:VPAD], 0.0, None, OP.mult)
                    for tt in range(16):
                        # deadmask columns for this token tile (col 64+65*h)
                        dcols = bass.AP(vv.tensor,
                                        vv.offset + tt * VT + 64,
                                        [list(vv.ap[0]), [VW, 16]])
                        nc.vector.tensor_scalar(
                            dcols, onesbf[:, 0:16], dead[:, tt:tt + 1], None,
                            OP.mult)
                    for nh in range(2):
                        wvh = wvp.tile([P, 8 * NQ], bf16, name="wvh")
                        load_fp(wvh[:], wv_d[:, nh * NQ:(nh + 1) * NQ], 8)
                        for tt in range(16):
                            xset = xbp.tile([P, 8 * P], bf16, name="xset")
                            load_fp(xset[:], xkvbf_d[:, tt * P:(tt + 1) * P], 8)
                            ps = pp.tile([P, NQ], f32, name="psv")
                            for f in range(8):
                                nc.tensor.matmul(
                                    ps[:], r(xset[:, f * P:(f + 1) * P]),
                                    r(wvh[:, f * NQ:(f + 1) * NQ]),
                                    start=(f == 0), stop=(f == 7))
                            # evict with deadmask, strided per-head layout
                            # psum col c (feat nh*512+c) -> vv col 65*(c//64)+(c%64)
                            vap = bass.AP(vv.tensor,
                                          vv.offset + tt * VT + nh * 8 * VW,
                                          [list(vv.ap[0]), [VW, 8], [1, 64]])
                            nc.vector.tensor_scalar(
                                vap, ps[:], dead[:, tt:tt + 1], None, OP.mult)

                # ---- phase 2: attention ----
                with tc.tile_pool(name="attn_p", bufs=4) as apool, \
                     tc.tile_pool(name="tri_p", bufs=1) as trip, \
                     tc.tile_pool(name="rec_p", bufs=2) as rpool, \
                     tc.tile_pool(name="ps2s", bufs=3, space="PSUM") as pps, \
                     tc.tile_pool(name="ps2a", bufs=3, space="PSUM") as ppa, \
                     tc.tile_pool(name="ps2b", bufs=2, space="PSUM") as ppb:
                    tri = trip.tile([P, 4 * NQ], bf16)
                    nc.sync.dma_start(tri[:], tri_d[:])
                    for h in range(H):
                        par, ht = h % 2, h // 2
                        pb = par * 64
                        ps_av = ppa.tile([P, NQ], f32, name="ps_av")
                        for kt in range(16):
                            ps_s = pps.tile([P, NQ], f32, name="ps_s")
                            nc.tensor.matmul(
                                ps_s[:],
                                r(kT[pb:pb + 64, ht * S + kt * P: ht * S + (kt + 1) * P]),
                                r(qT[pb:pb + 64, ht * NQ:(ht + 1) * NQ]),
                                start=True, stop=True)
                            attn = apool.tile([P, NQ], bf16, name="attn")
                            nc.scalar.activation(attn[:], ps_s[:], AF.Exp,
                                                 scale=0.125)
                            if kt < 4:
                                nc.vector.tensor_mul(
                                    attn[:], attn[:],
                                    tri[:, kt * NQ:(kt + 1) * NQ])
                            lhs = vv[:, kt * VT + VW * h: kt * VT + VW * h + VW]
                            nc.tensor.matmul(ps_av[0:VW, :], r(lhs), r(attn[:]),
                                             start=(kt == 0), stop=(kt == 15),
                                             skip_group_check=True)
                        # data rows 0..63, sumexp row 64 (all heads); odd heads
                        # are placed at outT rows 64..127 via a shifted DVE write
                        recip = rpool.tile([P, NQ], f32r, name="recip")
                        with nc.allow_low_precision(reason="f32r==f32 bits"):
                            nc.vector.reciprocal(recip[64:65, :],
                                                 ps_av[64:65, :])
                        ps_b = ppb.tile([P, NQ], f32, name="ps_b")
                        nc.tensor.matmul(ps_b[0:64, :],
                                         r(ones[64:65, 0:64]),
                                         r(recip[64:65, :]),
                                         start=True, stop=True)
                        rb = rpool.tile([P, NQ], f32, name="rb", tag="rb")
                        nc.scalar.copy(rb[0:64, :], ps_b[0:64, :])
                        oslice = outT[pb:pb + 64, ht * NQ:(ht + 1) * NQ]
                        nc.vector.tensor_mul(oslice, ps_av[0:64, :],
                                             rb[0:64, :])
                        nc.vector.tensor_scalar(
                            oslice, oslice,
                            biases["bv"][pb:pb + 64, h // 2: h // 2 + 1],
                            None, OP.add)

            # ---- phase 3: O-proj + LN1 + FFN + LN2 ----
            with tc.tile_pool(name="f3", bufs=1) as f3, \
                 tc.tile_pool(name="w3_p", bufs=3) as wp3, \
                 tc.tile_pool(name="sq_p", bufs=2) as sqp, \
                 tc.tile_pool(name="stat_p", bufs=1) as stp, \
                 tc.tile_pool(name="psmm", bufs=2, space="PSUM") as pmm, \
                 tc.tile_pool(name="psst", bufs=2, space="PSUM") as pst, \
                 tc.tile_pool(name="psbc", bufs=2, space="PSUM") as pbc:
                xq = f3.tile([P, 8 * NQ], f32)
                for m in range(8):
                    nc.sync.dma_start(xq[:, m * NQ:(m + 1) * NQ],
                                      xqT_d[m * P:(m + 1) * P, :])
                x1 = f3.tile([P, 8 * NQ], f32r)
                hh = f3.tile([P, 32 * NQ], f32r)
                x2 = f3.tile([P, 8 * NQ], f32r)

                # O-projection + residual
                for m in range(8):
                    wot = wp3.tile([P, 8 * P], f32r, name="wot", tag="wsmall")
                    load_fp(wot[:], wo_d[:, m * P:(m + 1) * P], 8)
                    ps = pmm.tile([P, NQ], f32, name="psmm")
                    for f2 in range(8):
                        nc.tensor.matmul(
                            ps[:], r(wot[:, f2 * P:(f2 + 1) * P]),
                            r(outT[:, f2 * NQ:(f2 + 1) * NQ]),
                            start=(f2 == 0), stop=(f2 == 7))
                    nc.vector.scalar_tensor_tensor(
                        x1[:, m * NQ:(m + 1) * NQ], ps[:],
                        biases["bo"][:, m:m + 1],
                        xq[:, m * NQ:(m + 1) * NQ], OP.add, OP.add)

                def layer_norm(xt, gname, bname):
                    ps_sum = pst.tile([1, NQ], f32, name="ps_sum", tag="sum")
                    ps_sq = pst.tile([1, NQ], f32, name="ps_sq", tag="sumsq")
                    for m in range(8):
                        nc.tensor.matmul(ps_sum[:], r(ones[:, 0:1]),
                                         r(xt[:, m * NQ:(m + 1) * NQ]),
                                         start=(m == 0), stop=(m == 7),
                                         skip_group_check=True)
                        sq = sqp.tile([P, NQ], f32r, name="sq")
                        nc.scalar.activation(sq[:], xt[:, m * NQ:(m + 1) * NQ],
                                             AF.Square)
                        nc.tensor.matmul(ps_sq[:], r(ones[:, 0:1]), r(sq[:]),
                                         start=(m == 0), stop=(m == 7),
                                         skip_group_check=True)
                    mu = stp.tile([1, NQ], f32r, name="mu")
                    nc.scalar.activation(mu[:], ps_sum[:], AF.Copy,
                                         scale=1.0 / D)
                    ex2 = stp.tile([1, NQ], f32, name="ex2")
                    nc.scalar.activation(ex2[:], ps_sq[:], AF.Copy,
                                         scale=1.0 / D)
                    var = stp.tile([1, NQ], f32, name="var")
                    nc.vector.scalar_tensor_tensor(
                        var[:], mu[:], 0.0, mu[:], OP.add, OP.mult)
                    # var = (ex2 + EPS) - mu^2
                    nc.vector.scalar_tensor_tensor(
                        var[:], ex2[:], EPS, var[:], OP.add, OP.subtract)
                    std = stp.tile([1, NQ], f32, name="std")
                    nc.scalar.activation(std[:], var[:], AF.Sqrt, bias=0.0)
                    rstd = stp.tile([1, NQ], f32r, name="rstd")
                    with nc.allow_low_precision(reason="f32r==f32 bits"):
                        nc.vector.reciprocal(rstd[:], std[:])
                    ps_mu = pbc.tile([P, NQ], f32, name="ps_mu", tag="bc")
                    nc.tensor.matmul(ps_mu[:], r(ones[0:1, :]), r(mu[:]),
                                     start=True, stop=True)
                    ps_rs = pbc.tile([P, NQ], f32, name="ps_rs", tag="bc")
                    nc.tensor.matmul(ps_rs[:], r(ones[0:1, :]), r(rstd[:]),
                                     start=True, stop=True)
                    for m in range(8):
                        sl = xt[:, m * NQ:(m + 1) * NQ]
                        nc.vector.tensor_sub(sl, sl, ps_mu[:])
                        nc.vector.tensor_mul(sl, sl, ps_rs[:])
                        nc.scalar.activation(sl, sl, AF.Identity,
                                             bias=biases[bname][:, m:m + 1],
                                             scale=biases[gname][:, m:m + 1])

                layer_norm(x1, "g1", "be1")

                # FFN1 (relu) -> hh
                for mf in range(32):
                    w1t = wp3.tile([P, 8 * P], f32r, name="w1t", tag="wsmall")
                    load_fp(w1t[:], w1_d[:, mf * P:(mf + 1) * P], 8)
                    ps = pmm.tile([P, NQ], f32, name="psmm")
                    for f in range(8):
                        nc.tensor.matmul(
                            ps[:], r(w1t[:, f * P:(f + 1) * P]),
                            r(x1[:, f * NQ:(f + 1) * NQ]),
                            start=(f == 0), stop=(f == 7))
                    nc.scalar.activation(hh[:, mf * NQ:(mf + 1) * NQ], ps[:],
                                         AF.Relu, bias=biases["b1"][:, mf:mf + 1])

                # FFN2 + residual -> x2
                for m in range(8):
                    w2t = wp3.tile([P, 32 * P], f32r, name="w2t", tag="wbig", bufs=2)
                    load_fp(w2t[:], w2_d[:, m * P:(m + 1) * P], 32)
                    ps = pmm.tile([P, NQ], f32, name="psmm")
                    for kf in range(32):
                        nc.tensor.matmul(
                            ps[:], r(w2t[:, kf * P:(kf + 1) * P]),
                            r(hh[:, kf * NQ:(kf + 1) * NQ]),
                            start=(kf == 0), stop=(kf == 31))
                    nc.vector.scalar_tensor_tensor(
                        x2[:, m * NQ:(m + 1) * NQ], ps[:],
                        biases["b2"][:, m:m + 1],
                        x1[:, m * NQ:(m + 1) * NQ], OP.add, OP.add)

                layer_norm(x2, "g2", "be2")

                for m in range(8):
                    nc.sync.dma_start(outT_d[m * P:(m + 1) * P, :],
                                      x2[:, m * NQ:(m + 1) * NQ])

    nc.compile()
    return nc


_CACHE = {}


def make_in_maps(inputs):
    x = np.asarray(inputs['x'], dtype=np.float32)
    shared = {}
    import ml_dtypes
    bf = ml_dtypes.bfloat16
    for nm in ("wq", "wk", "wo", "w1", "w2"):
        shared[nm] = np.ascontiguousarray(np.asarray(inputs[nm], np.float32))
    shared["wv"] = np.ascontiguousarray(np.asarray(inputs["wv"], np.float32).astype(bf))
    for nm, w in [("bq", 8), ("bk", 8), ("bv", 8), ("bo", 8), ("b2", 8)]:
        shared[nm + "_pt"] = np.ascontiguousarray(
            np.asarray(inputs[nm], np.float32).reshape(w, P).T)
    shared["b1_pt"] = np.ascontiguousarray(
        np.asarray(inputs["b1"], np.float32).reshape(32, P).T)
    for src, dst in [("g1", "g1_pt"), ("beta1", "beta1_pt"),
                     ("g2", "g2_pt"), ("beta2", "beta2_pt")]:
        shared[dst] = np.ascontiguousarray(
            np.asarray(inputs[src], np.float32).reshape(8, P).T)
    shared["onesmat"] = np.ones((P, P), np.float32)
    shared["onesbf"] = np.ones((P, P), bf)
    tri = np.zeros((P, 4 * NQ), np.float32)
    for kt in range(4):
        k_loc = kt * P + np.arange(P)[:, None]
        q = np.arange(NQ)[None, :]
        tri[:, kt * NQ:(kt + 1) * NQ] = (k_loc <= q).astype(np.float32)
    shared["trimask"] = tri.astype(bf)

    in_maps = []
    for c in range(NCORES):
        b, j = c // 4, c % 4
        xb = x[b]                                  # [S, D]
        perm = np.concatenate([np.arange(NQ * j, NQ * (j + 1)),
                               np.arange(0, NQ * j),
                               np.arange(NQ * (j + 1), S)])
        m = dict(shared)
        m["xkvT"] = np.ascontiguousarray(xb.T[:, perm])
        m["xkv_bf"] = m["xkvT"].astype(bf)
        m["xqT"] = np.ascontiguousarray(xb.T[:, NQ * j:NQ * (j + 1)])
        ndead = S - (NQ + NQ * j)
        dead = np.concatenate([np.ones(NQ + NQ * j, np.float32),
                               np.zeros(ndead, np.float32)])
        m["deadmask"] = np.ascontiguousarray(dead.reshape(16, P).T)
        in_maps.append(m)
    return in_maps


def kernel(**inputs):
    from concourse.bass_utils import run_bass_kernel_spmd
    if "nc" not in _CACHE:
        _CACHE["nc"] = build_nc()
    nc = _CACHE["nc"]
    in_maps = make_in_maps(inputs)
    res = run_bass_kernel_spmd(nc, in_maps, core_ids=list(range(NCORES)))
    out = np.empty((B, S, D), np.float32)
    for c in range(NCORES):
        b, j = c // 4, c % 4
        out[b, NQ * j:NQ * (j + 1), :] = res.results[c]["outT"].T
    return out
